# revision 19
# baseline (speedup 1.0000x reference)
"""Trainium2 Bass kernel for the B-spline (KAN-style) layer:

    out = einsum('bin,ion->bo', b_splines(tanh(x)), coeffs) + x @ base_weight

The 11-dim cubic-spline space (7 interior knots at t = +-0.25k) is expressed
in a device-cheap dictionary: {1, x, t, t^2, t^3, |t-s|^3 for knots s}, fit by
weighted least squares in L2 of the data measure t = tanh(N(0,1)) (exact for
the full 7-knot set; the fit matrix folds into the coefficient tensor on
host).  The whole layer is then ONE matmul with contraction K = NP*1024.

Device mapping (per core, data-parallel over batch, 8 cores x 512 rows):
  - planes (stationary, bf16 [128,128] slices) x weights (moving, bf16
    [128,1024]) -> PSUM [128 batch, 1024 out] f32, 4 PSUM tiles = 8 banks.
  - moving N=1024 (bf16) halves instruction count vs f32r N=512 and FWL
    halves LDWEIGHTS; output comes out batch-major (no host transpose).
"""
import numpy as np
import ml_dtypes

import concourse.bass as bass
import concourse.mybir as mybir
import concourse.tile as tile
from concourse import bacc, bass_utils
from concourse.bass_interp import get_hw_module

B, F, O, NCTRL = 4096, 1024, 1024, 11
NCORES = 8
BS = B // NCORES          # 512 batch rows per core
P = 128
FT = F // P               # 8 feature tiles
BT = BS // P              # 4 batch sub-tiles
F32 = mybir.dt.float32
F32R = mybir.dt.float32r
BF16 = mybir.dt.bfloat16
ACTF = mybir.ActivationFunctionType

# Plane dictionary (beyond const + x): polynomials t^1..t^DPOLY and
# |t - s|^3 kink planes.  The full set below spans the spline space exactly.
DPOLY = 3
KNOTS = (-0.75, -0.5, -0.25, 0.0, 0.25, 0.5, 0.75)
NP = 1 + DPOLY + len(KNOTS)   # x + polys + kinks = matmul planes
KT = NP * FT                  # k-tiles
NWARM = 10

_cached_program = None
_cached_fit = None


def _b_splines_np(t, grid, order=3):
    te = t[..., None]
    basis = ((te >= grid[:-1]) & (te < grid[1:])).astype(np.float64)
    for k in range(1, order + 1):
        ld = grid[k:-1] - grid[:-k - 1]
        ld = np.where(ld == 0, 1.0, ld)
        left = (te - grid[:-k - 1]) / ld * basis[..., :-1]
        rd = grid[k + 1:] - grid[1:-k]
        rd = np.where(rd == 0, 1.0, rd)
        right = (grid[k + 1:] - te) / rd * basis[..., 1:]
        basis = left + right
    return basis


def _fit_A():
    """L2(tanh-gaussian) projection of the 11 b-spline basis functions onto
    the device dictionary [1, z, t..t^DPOLY, |t-s|^3...].  Returns A
    [2+DPOLY+len(KNOTS), 11]; exact (residual ~1e-27) for the full knot set."""
    global _cached_fit
    if _cached_fit is not None:
        return _cached_fit
    z = np.linspace(-6.5, 6.5, 200001)
    w = np.exp(-z * z / 2)
    w /= w.sum()
    t = np.tanh(z)
    grid = np.linspace(-1.75, 1.75, 15)
    T = _b_splines_np(t, grid)                      # [NZ, 11]
    cols = [np.ones_like(t), z]
    for k in range(1, DPOLY + 1):
        cols.append(t ** k)
    for s in KNOTS:
        cols.append(np.abs(t - s) ** 3)
    D = np.stack(cols, 1)
    sw = np.sqrt(w)[:, None]
    A, *_ = np.linalg.lstsq(D * sw, T * sw, rcond=None)
    _cached_fit = A
    return A


def _precompute_weights(coeffs, base_weight):
    """Fold the dictionary fit into the coefficient tensor.
    Returns wk [NP*F, O] bf16 (plane-block order: x, t..t^DPOLY, kinks),
    and biasrep [P, O] f32 (const plane replicated across partitions)."""
    A = _fit_A()                                    # [2+DPOLY+NK, 11]
    c = coeffs.astype(np.float64)
    V = np.einsum("qn,fon->qfo", A, c)              # [ncol, F, O]
    bias = V[0].sum(axis=0)                         # [O]
    W0 = base_weight.astype(np.float64) + V[1]      # x plane
    blocks = [W0] + [V[2 + i] for i in range(DPOLY + len(KNOTS))]
    wk = np.concatenate(blocks, axis=0)             # [NP*F, O]
    bias1 = np.ascontiguousarray(bias.astype(np.float32).reshape(1, O))
    return np.ascontiguousarray(wk.astype(np.float32)), bias1


def _build_program():
    nc = bacc.Bacc("TRN2", target_bir_lowering=False, debug=False,
                   enable_asserts=False, num_devices=NCORES)
    # const APs for float biases used by scalar.activation(Abs, bias=-s)
    for s in KNOTS:
        if s != 0.0:
            v = float(-s)
            ct = nc.alloc_sbuf_tensor(f"const-float32-{v}", [P, 1], F32)
            nc.gpsimd.memset(ct.ap(), v)
            nc.const_aps.aps[(F32, v)] = ct.ap()
    nc.all_engine_barrier()

    xt_d = nc.dram_tensor("xt", [F, BS], F32R, kind="ExternalInput").ap()
    wk_d = nc.dram_tensor("wk", [NP * F, O], F32R, kind="ExternalInput").ap()
    bias_d = nc.dram_tensor("bias", [1, O], F32R, kind="ExternalInput").ap()
    out_d = nc.dram_tensor("out", [BS, O], F32, kind="ExternalOutput").ap()

    with tile.TileContext(nc) as tc:
        with tc.tile_pool(name="const", bufs=1) as const_pool, \
             tc.tile_pool(name="tpool", bufs=1) as t_pool, \
             tc.tile_pool(name="qpool", bufs=3) as q_pool, \
             tc.tile_pool(name="ppool", bufs=6) as p_pool, \
             tc.tile_pool(name="wpool", bufs=8) as w_pool, \
             tc.tile_pool(name="epool", bufs=4) as e_pool, \
             tc.tile_pool(name="psum", bufs=1, space="PSUM") as psum_pool:

            # input DMAs up front (gpsimd queue; sync queue leads with wk)
            xts = []
            for f in range(FT):
                xt = t_pool.tile([P, BS], F32R, tag=f"xt{f}", name=f"xt{f}")
                nc.gpsimd.dma_start(xt[:], xt_d[f * P:(f + 1) * P, :])
                xts.append(xt)
            bias_t = const_pool.tile([1, O], F32R)
            nc.gpsimd.dma_start(bias_t[:], bias_d)
            ones_f = const_pool.tile([1, P], F32)
            nc.gpsimd.memset(ones_f[:], 1.0)
            ones_t = const_pool.tile([1, P], F32R)
            nc.vector.tensor_copy(ones_t[:], ones_f[:])

            psums = [psum_pool.tile([P, O], F32, tag=f"ps{b}", name=f"ps{b}")
                     for b in range(BT)]

            # HAM warmup: garbage f32r matmuls while the first DMAs land.
            warmf = const_pool.tile([P, BS], F32)
            nc.vector.memset(warmf[:], 0.0)
            warm = const_pool.tile([P, BS], F32R)
            nc.vector.tensor_copy(warm[:], warmf[:])
            for i in range(NWARM):
                nc.tensor.matmul(psums[i % BT][:, 0:BS], warm[:, 0:P], warm[:],
                                 start=True, stop=True, skip_group_check=True)

            # bias init: psum[b][i, :] = ones.T @ bias = bias (K=1 matmul,
            # f32r so N<=512 per instruction). start=True opens the real
            # accumulation group; the plane matmuls accumulate on top.
            for b in range(BT):
                for h in range(2):
                    nc.tensor.matmul(
                        psums[b][:, h * 512:(h + 1) * 512], ones_t[:],
                        bias_t[:, h * 512:(h + 1) * 512],
                        start=True, stop=False, skip_group_check=True)

            # t = tanh(x) per feature tile (f32, kept resident)
            ts_ = []
            for f in range(FT):
                tt = t_pool.tile([P, BS], F32, tag=f"t{f}", name=f"t{f}")
                nc.scalar.activation(tt[:], xts[f][:].bitcast(F32), ACTF.Tanh)
                ts_.append(tt)
            # t^2 (f32, resident: feeds t^2 plane and t^3)
            t2s = []
            if DPOLY >= 2:
                for f in range(FT):
                    t2 = t_pool.tile([P, BS], F32, tag=f"t2{f}", name=f"t2{f}")
                    nc.scalar.activation(t2[:], ts_[f][:], ACTF.Square)
                    t2s.append(t2)

            def make_plane(p, f):
                """Emit ops producing plane (p, f) as an f32r [P, BS] tile."""
                if p == 0:          # x plane: raw input tile, no compute
                    return xts[f]
                pl = p_pool.tile([P, BS], F32R, tag="plane", name=f"pl{p}_{f}")
                if p == 1:          # t
                    nc.vector.tensor_copy(pl[:], ts_[f][:])
                elif p == 2 and DPOLY >= 2:   # t^2
                    nc.gpsimd.tensor_copy(pl[:], t2s[f][:])
                elif p == 3 and DPOLY >= 3:   # t^3
                    nc.vector.tensor_mul(pl[:], t2s[f][:], ts_[f][:])
                elif p <= DPOLY:    # t^4, t^5 if configured
                    q = q_pool.tile([P, BS], F32, tag="q4", name=f"t4_{f}")
                    nc.scalar.activation(q[:], t2s[f][:], ACTF.Square)
                    if p == 4:
                        nc.vector.tensor_copy(pl[:], q[:])
                    else:
                        nc.vector.tensor_mul(pl[:], q[:], ts_[f][:])
                else:               # |t - s|^3
                    s = KNOTS[p - 1 - DPOLY]
                    q = q_pool.tile([P, BS], F32, tag="q", name=f"q{p}_{f}")
                    if s == 0.0:
                        nc.scalar.activation(q[:], ts_[f][:], ACTF.Abs)
                    else:
                        nc.scalar.activation(q[:], ts_[f][:], ACTF.Abs,
                                             bias=float(-s))
                    q2 = q_pool.tile([P, BS], F32, tag="q2", name=f"q2_{p}_{f}")
                    nc.scalar.activation(q2[:], q[:], ACTF.Square)
                    nc.vector.tensor_mul(pl[:], q2[:], q[:])
                return pl

            for kt in range(KT):
                p, f = divmod(kt, FT)
                pl = make_plane(p, f)
                wt = w_pool.tile([P, O], F32R, tag="wk", name=f"wk{kt}")
                nc.sync.dma_start(wt[:], wk_d[kt * P:(kt + 1) * P, :])
                for b in range(BT):
                    for h in range(2):
                        nc.tensor.matmul(psums[b][:, h * 512:(h + 1) * 512],
                                         pl[:, b * P:(b + 1) * P],
                                         wt[:, h * 512:(h + 1) * 512],
                                         start=False, stop=(kt == KT - 1),
                                         skip_group_check=(kt == 0))

            # evict: bias is already in PSUM, so this is a pure copy,
            # split ACT/DVE; out-DMAs split across queues
            for b in range(BT):
                ot = e_pool.tile([P, O], F32, tag=f"evict{b}", name=f"ev{b}")
                if b % 2 == 0:
                    nc.scalar.copy(ot[:], psums[b][:])
                else:
                    nc.vector.tensor_copy(ot[:], psums[b][:])
                deng = (nc.sync, nc.gpsimd, nc.scalar)[b % 3]
                deng.dma_start(out_d[b * P:(b + 1) * P, :], ot[:])

    nc.compile()
    nc.m = get_hw_module(nc.m)
    return nc


def kernel(x, coeffs, base_weight, grid):
    global _cached_program
    x = np.asarray(x, np.float32)
    coeffs = np.asarray(coeffs, np.float32)
    base_weight = np.asarray(base_weight, np.float32)

    wk, bias1 = _precompute_weights(coeffs, base_weight)
    if _cached_program is None:
        _cached_program = _build_program()
    nc = _cached_program

    in_maps = []
    for c in range(NCORES):
        xs = np.ascontiguousarray(x[c * BS:(c + 1) * BS, :].T)  # [F, BS]
        in_maps.append({"xt": xs, "wk": wk, "bias": bias1})

    res = bass_utils.run_bass_kernel_spmd(nc, in_maps, core_ids=list(range(NCORES)))
    out = np.empty((B, O), np.float32)
    for c in range(NCORES):
        out[c * BS:(c + 1) * BS, :] = res.results[c]["out"]
    return out


# revision 22
# speedup vs baseline: 1.1642x; 1.1642x over previous
"""Trainium2 Bass kernel for the B-spline (KAN-style) layer:

    out = einsum('bin,ion->bo', b_splines(tanh(x)), coeffs) + x @ base_weight

The 11-dim cubic-spline space (7 interior knots at t = +-0.25k) is expressed
in a device-cheap dictionary: {1, x, t, t^2, t^3, |t-s|^3 for knots s}, fit by
weighted least squares in L2 of the data measure t = tanh(N(0,1)) (exact for
the full 7-knot set; the fit matrix folds into the coefficient tensor on
host).  The whole layer is then ONE matmul with contraction K = NP*1024.

Device mapping (per core, data-parallel over batch, 8 cores x 512 rows):
  - planes (stationary, bf16 [128,128] slices) x weights (moving, bf16
    [128,1024]) -> PSUM [128 batch, 1024 out] f32, 4 PSUM tiles = 8 banks.
  - moving N=1024 (bf16) halves instruction count vs f32r N=512 and FWL
    halves LDWEIGHTS; output comes out batch-major (no host transpose).
"""
import numpy as np
import ml_dtypes

import concourse.bass as bass
import concourse.mybir as mybir
import concourse.tile as tile
from concourse import bacc, bass_utils
from concourse.bass_interp import get_hw_module

B, F, O, NCTRL = 4096, 1024, 1024, 11
NCORES = 8
BS = B // NCORES          # 512 batch rows per core
P = 128
FT = F // P               # 8 feature tiles
BT = BS // P              # 4 batch sub-tiles
F32 = mybir.dt.float32
F32R = mybir.dt.float32r
BF16 = mybir.dt.bfloat16
ACTF = mybir.ActivationFunctionType

# Plane dictionary (beyond const + x): polynomials t^1..t^DPOLY and
# |t - s|^3 kink planes.  The full set below spans the spline space exactly.
DPOLY = 3
KNOTS = (-0.75, -0.5, -0.25, 0.0, 0.25, 0.5, 0.75)
NP = 1 + DPOLY + len(KNOTS)   # x + polys + kinks = matmul planes
KT = NP * FT                  # k-tiles
NWARM = 10

_cached_program = None
_cached_fit = None


def _b_splines_np(t, grid, order=3):
    te = t[..., None]
    basis = ((te >= grid[:-1]) & (te < grid[1:])).astype(np.float64)
    for k in range(1, order + 1):
        ld = grid[k:-1] - grid[:-k - 1]
        ld = np.where(ld == 0, 1.0, ld)
        left = (te - grid[:-k - 1]) / ld * basis[..., :-1]
        rd = grid[k + 1:] - grid[1:-k]
        rd = np.where(rd == 0, 1.0, rd)
        right = (grid[k + 1:] - te) / rd * basis[..., 1:]
        basis = left + right
    return basis


def _fit_A():
    """L2(tanh-gaussian) projection of the 11 b-spline basis functions onto
    the device dictionary [1, z, t..t^DPOLY, |t-s|^3...].  Returns A
    [2+DPOLY+len(KNOTS), 11]; exact (residual ~1e-27) for the full knot set."""
    global _cached_fit
    if _cached_fit is not None:
        return _cached_fit
    z = np.linspace(-6.5, 6.5, 200001)
    w = np.exp(-z * z / 2)
    w /= w.sum()
    t = np.tanh(z)
    grid = np.linspace(-1.75, 1.75, 15)
    T = _b_splines_np(t, grid)                      # [NZ, 11]
    cols = [np.ones_like(t), z]
    for k in range(1, DPOLY + 1):
        cols.append(t ** k)
    for s in KNOTS:
        cols.append(np.abs(t - s) ** 3)
    D = np.stack(cols, 1)
    sw = np.sqrt(w)[:, None]
    A, *_ = np.linalg.lstsq(D * sw, T * sw, rcond=None)
    _cached_fit = A
    return A


def _precompute_weights(coeffs, base_weight):
    """Fold the dictionary fit into the coefficient tensor.
    Returns wk [NP*F, O] bf16 (plane-block order: x, t..t^DPOLY, kinks),
    and biasrep [P, O] f32 (const plane replicated across partitions)."""
    A = _fit_A()                                    # [2+DPOLY+NK, 11]
    c = coeffs.astype(np.float64)
    V = np.einsum("qn,fon->qfo", A, c)              # [ncol, F, O]
    bias = V[0].sum(axis=0)                         # [O]
    W0 = base_weight.astype(np.float64) + V[1]      # x plane
    blocks = [W0] + [V[2 + i] for i in range(DPOLY + len(KNOTS))]
    wk = np.concatenate(blocks, axis=0)             # [NP*F, O]
    bias1 = np.ascontiguousarray(bias.astype(np.float32).reshape(1, O))
    return np.ascontiguousarray(wk.astype(np.float32)), bias1


def _build_program():
    nc = bacc.Bacc("TRN2", target_bir_lowering=False, debug=False,
                   enable_asserts=False, num_devices=NCORES)
    # const APs for float biases used by scalar.activation(Abs, bias=-s)
    for s in KNOTS:
        if s != 0.0:
            v = float(-s)
            ct = nc.alloc_sbuf_tensor(f"const-float32-{v}", [P, 1], F32)
            nc.gpsimd.memset(ct.ap(), v)
            nc.const_aps.aps[(F32, v)] = ct.ap()
    nc.all_engine_barrier()

    xt_d = nc.dram_tensor("xt", [F, BS], F32R, kind="ExternalInput").ap()
    wk_d = nc.dram_tensor("wk", [NP * F, O], F32R, kind="ExternalInput").ap()
    bias_d = nc.dram_tensor("bias", [1, O], F32R, kind="ExternalInput").ap()
    out_d = nc.dram_tensor("out", [BS, O], F32, kind="ExternalOutput").ap()

    with tile.TileContext(nc) as tc:
        with tc.tile_pool(name="const", bufs=1) as const_pool, \
             tc.tile_pool(name="tpool", bufs=1) as t_pool, \
             tc.tile_pool(name="qpool", bufs=3) as q_pool, \
             tc.tile_pool(name="ppool", bufs=6) as p_pool, \
             tc.tile_pool(name="wpool", bufs=8) as w_pool, \
             tc.tile_pool(name="epool", bufs=1) as e_pool, \
             tc.tile_pool(name="psum", bufs=1, space="PSUM") as psum_pool:

            # input DMAs up front (gpsimd queue; sync queue leads with wk)
            xts = []
            for f in range(FT):
                xt = t_pool.tile([P, BS], F32R, tag=f"xt{f}", name=f"xt{f}")
                nc.gpsimd.dma_start(xt[:], xt_d[f * P:(f + 1) * P, :])
                xts.append(xt)
            bias_t = const_pool.tile([1, O], F32R)
            nc.gpsimd.dma_start(bias_t[:], bias_d)
            ones_f = const_pool.tile([1, P], F32)
            nc.gpsimd.memset(ones_f[:], 1.0)
            ones_t = const_pool.tile([1, P], F32R)
            nc.vector.tensor_copy(ones_t[:], ones_f[:])

            # 8 single-bank PSUM tiles, indexed [b][h] (b = batch subtile,
            # h = output half)
            psums = [[psum_pool.tile([P, O // 2], F32, tag=f"ps{b}_{h}",
                                     name=f"ps{b}_{h}") for h in range(2)]
                     for b in range(BT)]

            # HAM warmup: garbage f32r matmuls while the first DMAs land.
            warmf = const_pool.tile([P, BS], F32)
            nc.vector.memset(warmf[:], 0.0)
            warm = const_pool.tile([P, BS], F32R)
            nc.vector.tensor_copy(warm[:], warmf[:])
            for i in range(NWARM):
                nc.tensor.matmul(psums[i % BT][i % 2][:], warm[:, 0:P],
                                 warm[:], start=True, stop=True,
                                 skip_group_check=True)

            # bias init: psum[b][h][i, :] = ones.T @ bias-half (K=1 matmul).
            # start=True opens the real accumulation group; the plane
            # matmuls accumulate on top.
            for b in range(BT):
                for h in range(2):
                    nc.tensor.matmul(
                        psums[b][h][:], ones_t[:],
                        bias_t[:, h * 512:(h + 1) * 512],
                        start=True, stop=False, skip_group_check=True)

            # t = tanh(x) per feature tile (f32, kept resident)
            ts_ = []
            for f in range(FT):
                tt = t_pool.tile([P, BS], F32, tag=f"t{f}", name=f"t{f}")
                nc.scalar.activation(tt[:], xts[f][:].bitcast(F32), ACTF.Tanh)
                ts_.append(tt)
            # t^2 (f32, resident: feeds t^2 plane and t^3)
            t2s = []
            if DPOLY >= 2:
                for f in range(FT):
                    t2 = t_pool.tile([P, BS], F32, tag=f"t2{f}", name=f"t2{f}")
                    nc.scalar.activation(t2[:], ts_[f][:], ACTF.Square)
                    t2s.append(t2)

            def make_plane(p, f):
                """Emit ops producing plane (p, f) as an f32r [P, BS] tile."""
                if p == 0:          # x plane: raw input tile, no compute
                    return xts[f]
                pl = p_pool.tile([P, BS], F32R, tag="plane", name=f"pl{p}_{f}")
                if p == 1:          # t
                    nc.vector.tensor_copy(pl[:], ts_[f][:])
                elif p == 2 and DPOLY >= 2:   # t^2
                    nc.gpsimd.tensor_copy(pl[:], t2s[f][:])
                elif p == 3 and DPOLY >= 3:   # t^3
                    nc.vector.tensor_mul(pl[:], t2s[f][:], ts_[f][:])
                elif p <= DPOLY:    # t^4, t^5 if configured
                    q = q_pool.tile([P, BS], F32, tag="q4", name=f"t4_{f}")
                    nc.scalar.activation(q[:], t2s[f][:], ACTF.Square)
                    if p == 4:
                        nc.vector.tensor_copy(pl[:], q[:])
                    else:
                        nc.vector.tensor_mul(pl[:], q[:], ts_[f][:])
                else:               # |t - s|^3
                    s = KNOTS[p - 1 - DPOLY]
                    q = q_pool.tile([P, BS], F32, tag="q", name=f"q{p}_{f}")
                    if s == 0.0:
                        nc.scalar.activation(q[:], ts_[f][:], ACTF.Abs)
                    else:
                        nc.scalar.activation(q[:], ts_[f][:], ACTF.Abs,
                                             bias=float(-s))
                    q2 = q_pool.tile([P, BS], F32, tag="q2", name=f"q2_{p}_{f}")
                    nc.scalar.activation(q2[:], q[:], ACTF.Square)
                    nc.vector.tensor_mul(pl[:], q2[:], q[:])
                return pl

            for kt in range(KT):
                p, f = divmod(kt, FT)
                pl = make_plane(p, f)
                wt = w_pool.tile([P, O], F32R, tag="wk", name=f"wk{kt}")
                nc.sync.dma_start(wt[:], wk_d[kt * P:(kt + 1) * P, :])
                for b in range(BT):
                    for h in range(2):
                        nc.tensor.matmul(psums[b][h][:],
                                         pl[:, b * P:(b + 1) * P],
                                         wt[:, h * 512:(h + 1) * 512],
                                         start=False, stop=(kt == KT - 1),
                                         skip_group_check=(kt == 0))

            # evict: bias is already in PSUM, so this is a pure copy,
            # split ACT/DVE; out-DMAs split across queues
            for b in range(BT):
                for h in range(2):
                    ot = e_pool.tile([P, O // 2], F32, tag=f"evict{b}_{h}",
                                     name=f"ev{b}_{h}")
                    if (2 * b + h) % 2 == 0:
                        nc.scalar.copy(ot[:], psums[b][h][:])
                    else:
                        nc.vector.tensor_copy(ot[:], psums[b][h][:])
                    deng = (nc.sync, nc.gpsimd, nc.scalar)[(2 * b + h) % 3]
                    deng.dma_start(
                        out_d[b * P:(b + 1) * P, h * 512:(h + 1) * 512],
                        ot[:])

    nc.compile()
    nc.m = get_hw_module(nc.m)
    return nc


def kernel(x, coeffs, base_weight, grid):
    global _cached_program
    x = np.asarray(x, np.float32)
    coeffs = np.asarray(coeffs, np.float32)
    base_weight = np.asarray(base_weight, np.float32)

    wk, bias1 = _precompute_weights(coeffs, base_weight)
    if _cached_program is None:
        _cached_program = _build_program()
    nc = _cached_program

    in_maps = []
    for c in range(NCORES):
        xs = np.ascontiguousarray(x[c * BS:(c + 1) * BS, :].T)  # [F, BS]
        in_maps.append({"xt": xs, "wk": wk, "bias": bias1})

    res = bass_utils.run_bass_kernel_spmd(nc, in_maps, core_ids=list(range(NCORES)))
    out = np.empty((B, O), np.float32)
    for c in range(NCORES):
        out[c * BS:(c + 1) * BS, :] = res.results[c]["out"]
    return out
